# revision 1
# baseline (speedup 1.0000x reference)
"""CenterLossB kernel for 8 Trainium2 NeuronCores.

Data-parallel over the batch: each of the 8 cores processes 8192 rows of
feat/label/wei.  The loss

    own      = sum_i w_i * ||f_i - c_{l_i}||^2
    distocen = sum_i w_i * (||f_i - c_{(l_i+1)%3}||^2 + ||f_i - c_{(l_i+2)%3}||^2)
    out      = own * (1 + 1/distocen) / 2 / B

is rewritten algebraically so the device only produces small per-core
partials (centers never need to go to the device):

    A       = sum_i w_i ||f_i||^2                  (fp32, DVE fused mul+reduce)
    V[k,:]  = sum_i w_i 1[l_i=k] f_i   k=0..2      (PE matmul, PSUM fp32 accum)
    V[3,:]  = sum_i w_i f_i
    wsum[k] = sum_i w_i 1[l_i=k],  wsum[3] = sum_i w_i

Host combine (float64):
    own   = A - 2*sum_k c_k.V[k] + sum_k ||c_k||^2 wsum[k]
    total = 3A - 2*(sum_k c_k).V[3] + (sum_k ||c_k||^2) * wsum[3]
    distocen = total - own
"""

import os
from contextlib import ExitStack

import numpy as np

import concourse.bass as bass
import concourse.bacc as bacc
import concourse.tile as tile
from concourse import mybir
from concourse.bass_utils import run_bass_kernel_spmd

N_CORES = 8
B = 65536
D = 512
BC = B // N_CORES          # 8192 rows per core
P = 128                    # SBUF partitions
NT = BC // P               # 64 sub-tiles of 128 rows
CHUNK = 4                  # sub-tiles per DMA chunk (4*128 rows * 2KB = 1 MiB)
NCHUNK = NT // CHUNK       # 16 chunks

# Results of the last device run (for test harnesses to inspect timing).
LAST_RESULTS = None

_COMPILED = {}


def _build(reps=1, chunk=CHUNK, bufs=16, alt_dma=False, sizes=None,
           dma_only=False, staggered=False, split_stt=False, io_rings=False,
           no_stt=False, no_vout=False, bf16_trash=False, tmajor=False):
    f32 = mybir.dt.float32
    f32r = mybir.dt.float32r
    i32 = mybir.dt.int32
    X = mybir.AxisListType.X
    op = mybir.AluOpType

    nc = bacc.Bacc("TRN2", target_bir_lowering=False, debug=False,
                   num_devices=N_CORES)

    # feat is declared float32r (same 32-bit layout as fp32; numpy side is
    # np.float32) so the full-rate fp32r matmul path passes BIR verification:
    # the verifier requires producers of fp32r matmul operands to be
    # fp32r-typed.  DVE consumers bitcast back to plain fp32.
    feat_d = nc.dram_tensor("feat", [BC, D], f32r, kind="ExternalInput")
    lab_d = nc.dram_tensor("label", [BC], i32, kind="ExternalInput")
    wei_d = nc.dram_tensor("wei", [BC], f32, kind="ExternalInput")
    V_d = nc.dram_tensor("V", [4, D], f32, kind="ExternalOutput")
    S_d = nc.dram_tensor("S", [5, 1], f32, kind="ExternalOutput")

    # row layout: global row r = p*NT + t  (p = partition, t = sub-tile idx)
    # tmajor (diagnostic, dma_only benches only): r = t*P + p, which makes each
    # chunk DMA one contiguous DRAM span instead of 128 scattered blocks
    if tmajor:
        assert dma_only, "tmajor needs wei/label transpose plumbing for compute"
        feat_r = feat_d.ap().rearrange("(t p) d -> p t d", p=P)
    else:
        feat_r = feat_d.ap().rearrange("(p t) d -> p t d", p=P)
    lab_r = lab_d.ap().rearrange("(p t) -> p t", p=P)
    wei_r = wei_d.ap().rearrange("(p t) -> p t", p=P)

    if sizes is None:
        sizes = (chunk,) * (NT // chunk)
    assert sum(sizes) == NT
    starts = [sum(sizes[:i]) for i in range(len(sizes))]
    max_chunk = max(sizes)

    with tile.TileContext(nc) as tc, ExitStack() as ctx:
        singles = ctx.enter_context(tc.tile_pool(name="singles", bufs=1))
        feat_pool = ctx.enter_context(tc.tile_pool(name="feat", bufs=bufs))
        psum = ctx.enter_context(tc.tile_pool(name="psum", bufs=1, space="PSUM"))

        # small I/O rides the second HWDGE ring (qActDynamicHW) so the feat
        # stream on qSPDynamicHW starts without queuing behind it
        io_eng = nc.scalar if io_rings else nc.sync
        w_all = singles.tile([P, NT], f32)
        io_eng.dma_start(w_all[:], wei_r)
        lab_i = singles.tile([P, NT], i32)
        io_eng.dma_start(lab_i[:], lab_r)
        lab_f = singles.tile([P, NT], f32)
        nc.vector.tensor_copy(lab_f[:], lab_i[:])

        # wk[p, k, t] = w * 1[label == k] for k=0..2; wk[p, 3, t] = w
        wk = singles.tile([P, 4, NT], f32)
        for k in range(3):
            nc.vector.scalar_tensor_tensor(
                out=wk[:, k, :], in0=lab_f[:], scalar=float(k), in1=w_all[:],
                op0=op.is_equal, op1=op.mult)
        nc.vector.tensor_copy(wk[:, 3, :], w_all[:])
        # bit-preserving copy into an fp32r-typed tile for the matmul lhsT
        wk_r = singles.tile([P, 4, NT], f32r)
        nc.vector.tensor_copy(wk_r[:], wk[:].bitcast(f32r))
        wk_t = wk_r.rearrange("p k t -> p t k")

        swsum = singles.tile([P, NT], f32)
        tdt = mybir.dt.bfloat16 if bf16_trash else f32
        trash = singles.tile([P, D], tdt)
        trash2 = singles.tile([P, D], tdt)
        ones = singles.tile([P, 1], f32)
        nc.vector.memset(ones[:], 1.0)

        def body():
            psum_V = None
            if not dma_only:
                psum_V = psum.tile([4, D], f32, tag="psum_V")
            for c, (t0, sz) in enumerate(zip(starts, sizes)):
                F = feat_pool.tile([P, max_chunk, D], f32r)
                eng = nc.scalar if (alt_dma and c % 2) else nc.sync
                eng.dma_start(F[:, :sz, :], feat_r[:, t0:t0 + sz, :])
                for a in range(sz if not dma_only else 0):
                    t = t0 + a
                    nc.tensor.matmul(
                        psum_V[:],
                        wk_t[:, t, :],
                        F[:, a, :],
                        start=(t == 0), stop=(t == NT - 1))
                    if no_stt:
                        continue
                    if split_stt and t % 2:
                        eng_v, tr = nc.gpsimd, trash2
                    else:
                        eng_v, tr = nc.vector, trash
                    eng_v.scalar_tensor_tensor(
                        out=tr[:], in0=F[:, a, :].bitcast(f32),
                        scalar=w_all[:, t:t + 1],
                        in1=F[:, a, :].bitcast(f32), op0=op.mult, op1=op.mult,
                        accum_out=swsum[:, t:t + 1])

            # epilogue: per-partition reductions, cross-partition via matmul
            rs = singles.tile([P, 5], f32)
            for k in range(4):
                nc.vector.tensor_reduce(out=rs[:, k:k + 1], in_=wk[:, k, :],
                                        axis=X, op=op.add)
            if dma_only or no_stt:
                nc.vector.memset(rs[:, 4:5], 0.0)
            else:
                nc.vector.tensor_reduce(out=rs[:, 4:5], in_=swsum[:], axis=X,
                                        op=op.add)
            psum_S = psum.tile([5, 1], f32)
            nc.tensor.matmul(psum_S[:], rs[:], ones[:])

            v_sb = singles.tile([4, D], f32)
            if dma_only or no_vout:
                nc.vector.memset(v_sb[:], 0.0)
            else:
                nc.vector.tensor_copy(v_sb[:], psum_V[:])
            nc.sync.dma_start(V_d.ap(), v_sb[:])
            s_sb = singles.tile([5, 1], f32)
            nc.vector.tensor_copy(s_sb[:], psum_S[:])
            io_eng.dma_start(S_d.ap(), s_sb[:])

        if reps == 1:
            body()
        else:
            with tc.For_i(0, reps, 1, staggered_reset=staggered):
                body()

    nc.compile()
    return nc


def _get_compiled(reps=1, **kw):
    key = (reps, tuple(sorted(kw.items())))
    if key not in _COMPILED:
        _COMPILED[key] = _build(reps, **kw)
    return _COMPILED[key]


def kernel(feat, label, wei, centers, batch_size):
    global LAST_RESULTS
    feat = np.ascontiguousarray(np.asarray(feat, dtype=np.float32))
    label = np.ascontiguousarray(np.asarray(label, dtype=np.int32))
    wei = np.ascontiguousarray(np.asarray(wei, dtype=np.float32))
    centers = np.asarray(centers, dtype=np.float32)
    bsz = float(np.asarray(batch_size))

    nc = _get_compiled()

    in_maps = []
    for i in range(N_CORES):
        sl = slice(i * BC, (i + 1) * BC)
        in_maps.append({
            "feat": feat[sl],
            "label": label[sl],
            "wei": wei[sl],
        })

    try:
        res = run_bass_kernel_spmd(nc, in_maps, list(range(N_CORES)))
    except ModuleNotFoundError:
        # BASS_TRACE was requested but this environment lacks the axon NTFF
        # profile hook (antenv.axon_hooks) — rerun without tracing.
        prev = os.environ.get("BASS_NEVER_TRACE")
        os.environ["BASS_NEVER_TRACE"] = "1"
        try:
            res = run_bass_kernel_spmd(nc, in_maps, list(range(N_CORES)))
        finally:
            if prev is None:
                os.environ.pop("BASS_NEVER_TRACE", None)
            else:
                os.environ["BASS_NEVER_TRACE"] = prev
    LAST_RESULTS = res

    # host combine in float64
    c = centers.astype(np.float64)            # [3, D]
    cn = (c * c).sum(axis=1)                  # ||c_k||^2
    csum = cn.sum()
    s_cent = c.sum(axis=0)                    # sum_k c_k

    A = 0.0
    B2 = 0.0
    T2 = 0.0
    wsum = np.zeros(4, dtype=np.float64)
    for r in res.results:
        V = r["V"].astype(np.float64)         # [4, D]
        S = r["S"].astype(np.float64).ravel() # [5]
        B2 += float((c * V[:3]).sum())
        T2 += float((s_cent * V[3]).sum())
        wsum += S[:4]
        A += float(S[4])

    own = A - 2.0 * B2 + float((cn * wsum[:3]).sum())
    total = 3.0 * A - 2.0 * T2 + csum * wsum[3]
    distocen = total - own
    out = own * (1.0 + 1.0 / distocen) / 2.0 / bsz
    return np.float32(out)



# revision 15
# speedup vs baseline: 1.2206x; 1.2206x over previous
"""CenterLossB kernel for 8 Trainium2 NeuronCores.

Data-parallel over the batch: each of the 8 cores processes 8192 rows of
feat/label/wei.  The loss

    own      = sum_i w_i * ||f_i - c_{l_i}||^2
    distocen = sum_i w_i * (||f_i - c_{(l_i+1)%3}||^2 + ||f_i - c_{(l_i+2)%3}||^2)
    out      = own * (1 + 1/distocen) / 2 / B

is rewritten algebraically so the device only produces small per-core
partials (centers never need to go to the device):

    A       = sum_i w_i ||f_i||^2                  (ACT Square + row accum)
    V[k,:]  = sum_i w_i 1[l_i=k] f_i   k=0..2      (PE matmul, PSUM fp32 accum)
    V[3,:]  = sum_i w_i f_i
    wsum[k] = sum_i w_i 1[l_i=k],  wsum[3] = sum_i w_i

Host combine (float64):
    own   = A - 2*sum_k c_k.V[k] + sum_k ||c_k||^2 wsum[k]
    total = 3A - 2*(sum_k c_k).V[3] + (sum_k ||c_k||^2) * wsum[3]
    distocen = total - own

Engine split while the feat stream DMAs in (4 MiB chunks tapering to
256 KiB so the post-stream tail is one sub-tile of compute):
    SP ring   : feat chunks              (the only traffic on qSPDynamicHW)
    ACT ring  : wei/label in, S out      (qActDynamicHW)
    PE        : V accumulation, then the 5-way scalar reduction
    ACT       : Square+accum per sub-tile (A term), V copy out of PSUM
    DVE       : prologue wk masks, tiny epilogue reductions
"""

import os
from contextlib import ExitStack

import numpy as np

import concourse.bass as bass
import concourse.bacc as bacc
import concourse.tile as tile
from concourse import mybir
from concourse.bass_utils import run_bass_kernel_spmd

N_CORES = 8
B = 65536
D = 512
BC = B // N_CORES          # 8192 rows per core
P = 128                    # SBUF partitions
NT = BC // P               # 64 sub-tiles of 128 rows

# DMA plan: sub-tiles per dma_start. Large chunks stream at peak HBM rate;
# the tapered tail keeps the last-landing data (and hence the post-stream
# compute tail) small.
SIZES = (16, 16, 8, 8, 8, 4, 2, 1, 1)

# Results of the last device run (for test harnesses to inspect timing).
LAST_RESULTS = None

_COMPILED = {}


def _build(reps=1, sizes=SIZES, bufs=5, stt="mix", dma_only=False,
           io_rings=True, out_ring="io"):
    f32 = mybir.dt.float32
    f32r = mybir.dt.float32r
    i32 = mybir.dt.int32
    X = mybir.AxisListType.X
    op = mybir.AluOpType
    AF = mybir.ActivationFunctionType

    nc = bacc.Bacc("TRN2", target_bir_lowering=False, debug=False,
                   num_devices=N_CORES)

    # feat is declared float32r (same 32-bit layout as fp32; numpy side is
    # np.float32) so the full-rate fp32r matmul path passes BIR verification:
    # the verifier requires producers of fp32r matmul operands to be
    # fp32r-typed.  Non-matmul consumers bitcast back to plain fp32.
    feat_d = nc.dram_tensor("feat", [BC, D], f32r, kind="ExternalInput")
    lab_d = nc.dram_tensor("label", [BC], i32, kind="ExternalInput")
    wei_d = nc.dram_tensor("wei", [BC], f32, kind="ExternalInput")
    # S[p, 0:NT]  = |f_{p,t}|^2 row sums (unweighted; host applies wei)
    # S[p, NT+k]  = sum_t wk[p, k, t]    (host sums over partitions)
    V_d = nc.dram_tensor("V", [4, D], f32, kind="ExternalOutput")
    S_d = nc.dram_tensor("S", [P, NT + 4], f32, kind="ExternalOutput")

    # row layout: global row r = p*NT + t  (p = partition, t = sub-tile idx)
    feat_r = feat_d.ap().rearrange("(p t) d -> p t d", p=P)
    lab_r = lab_d.ap().rearrange("(p t) -> p t", p=P)
    wei_r = wei_d.ap().rearrange("(p t) -> p t", p=P)

    assert sum(sizes) == NT
    starts = [sum(sizes[:i]) for i in range(len(sizes))]
    max_chunk = max(sizes)

    with tile.TileContext(nc) as tc, ExitStack() as ctx:
        singles = ctx.enter_context(tc.tile_pool(name="singles", bufs=1))
        feat_pool = ctx.enter_context(tc.tile_pool(name="feat", bufs=bufs))
        psum = ctx.enter_context(tc.tile_pool(name="psum", bufs=1, space="PSUM"))

        # small I/O rides the second HWDGE ring (qActDynamicHW) so the feat
        # stream on qSPDynamicHW starts without queuing behind it
        io_eng = nc.scalar if io_rings else nc.sync
        w_all = singles.tile([P, NT], f32)
        io_eng.dma_start(w_all[:], wei_r)
        lab_i = singles.tile([P, NT], i32)
        io_eng.dma_start(lab_i[:], lab_r)
        lab_f = singles.tile([P, NT], f32)
        nc.vector.tensor_copy(lab_f[:], lab_i[:])

        # wk[p, k, t] = w * 1[label == k] for k=0..2; wk[p, 3, t] = w
        wk = singles.tile([P, 4, NT], f32)
        for k in range(3):
            nc.vector.scalar_tensor_tensor(
                out=wk[:, k, :], in0=lab_f[:], scalar=float(k), in1=w_all[:],
                op0=op.is_equal, op1=op.mult)
        nc.vector.tensor_copy(wk[:, 3, :], w_all[:])
        # bit-preserving copy into an fp32r-typed tile for the matmul lhsT
        wk_r = singles.tile([P, 4, NT], f32r)
        nc.vector.tensor_copy(wk_r[:], wk[:].bitcast(f32r))
        wk_t = wk_r.rearrange("p k t -> p t k")

        trash = singles.tile([P, D], f32)
        trash2 = singles.tile([P, D], f32)

        # qs[:, 0:NT] = per-sub-tile |f|^2 row sums; qs[:, NT:] = wk sums.
        # The whole [P, NT+4] tile ships to the host in one DMA — no
        # cross-partition reduction on the device's critical path.
        qs = singles.tile([P, NT + 4], f32)
        for k in range(4):
            nc.vector.tensor_reduce(out=qs[:, NT + k:NT + k + 1],
                                    in_=wk[:, k, :], axis=X, op=op.add)

        # which engine squares sub-tile t (DVE takes odd t so the final
        # sub-tile's square runs on the cheaper engine)
        def on_dve(t):
            return stt == "dve" or (stt == "mix" and t % 2 == 1)

        out_eng = nc.sync if out_ring == "sp" else io_eng

        def body():
            psum_V = None
            if not dma_only:
                psum_V = psum.tile([4, D], f32, tag="psum_V")
            for c, (t0, sz) in enumerate(zip(starts, sizes)):
                F = feat_pool.tile([P, max_chunk, D], f32r)
                nc.sync.dma_start(F[:, :sz, :], feat_r[:, t0:t0 + sz, :])
                for a in range(sz if not dma_only else 0):
                    t = t0 + a
                    nc.tensor.matmul(
                        psum_V[:],
                        wk_t[:, t, :],
                        F[:, a, :],
                        start=(t == 0), stop=(t == NT - 1))
                    if on_dve(t):
                        nc.vector.scalar_tensor_tensor(
                            out=trash[:], in0=F[:, a, :].bitcast(f32),
                            scalar=1.0,
                            in1=F[:, a, :].bitcast(f32),
                            op0=op.mult, op1=op.mult,
                            accum_out=qs[:, t:t + 1])
                    else:
                        nc.scalar.activation(
                            out=trash2[:], in_=F[:, a, :].bitcast(f32),
                            func=AF.Square,
                            accum_out=qs[:, t:t + 1])

            if dma_only:
                nc.vector.memset(qs[:, 0:NT], 0.0)
            out_eng.dma_start(S_d.ap(), qs[:])

            v_sb = singles.tile([4, D], f32)
            if dma_only:
                nc.vector.memset(v_sb[:], 0.0)
            elif stt == "dve":
                nc.vector.tensor_copy(v_sb[:], psum_V[:])
            else:
                # DVE squares the final sub-tile; ACT drains V from PSUM
                nc.scalar.activation(out=v_sb[:], in_=psum_V[:], func=AF.Copy)
            # V rides SP (idle once the feat stream ends) while S rides the
            # ACT ring — the two output DMAs land concurrently
            nc.sync.dma_start(V_d.ap(), v_sb[:])

        if reps == 1:
            body()
        else:
            with tc.For_i(0, reps, 1):
                body()

    nc.compile()
    return nc


def _get_compiled(reps=1, **kw):
    key = (reps, tuple(sorted(kw.items())))
    if key not in _COMPILED:
        _COMPILED[key] = _build(reps, **kw)
    return _COMPILED[key]


def kernel(feat, label, wei, centers, batch_size):
    global LAST_RESULTS
    feat = np.ascontiguousarray(np.asarray(feat, dtype=np.float32))
    label = np.ascontiguousarray(np.asarray(label, dtype=np.int32))
    wei = np.ascontiguousarray(np.asarray(wei, dtype=np.float32))
    centers = np.asarray(centers, dtype=np.float32)
    bsz = float(np.asarray(batch_size))

    nc = _get_compiled()

    in_maps = []
    for i in range(N_CORES):
        sl = slice(i * BC, (i + 1) * BC)
        in_maps.append({
            "feat": feat[sl],
            "label": label[sl],
            "wei": wei[sl],
        })

    try:
        res = run_bass_kernel_spmd(nc, in_maps, list(range(N_CORES)))
    except ModuleNotFoundError:
        # BASS_TRACE was requested but this environment lacks the axon NTFF
        # profile hook (antenv.axon_hooks) — rerun without tracing.
        prev = os.environ.get("BASS_NEVER_TRACE")
        os.environ["BASS_NEVER_TRACE"] = "1"
        try:
            res = run_bass_kernel_spmd(nc, in_maps, list(range(N_CORES)))
        finally:
            if prev is None:
                os.environ.pop("BASS_NEVER_TRACE", None)
            else:
                os.environ["BASS_NEVER_TRACE"] = prev
    LAST_RESULTS = res

    # host combine in float64
    c = centers.astype(np.float64)            # [3, D]
    cn = (c * c).sum(axis=1)                  # ||c_k||^2
    csum = cn.sum()
    s_cent = c.sum(axis=0)                    # sum_k c_k
    A = 0.0
    B2 = 0.0
    T2 = 0.0
    wsum = np.zeros(4, dtype=np.float64)
    for i, r in enumerate(res.results):
        V = r["V"].astype(np.float64)         # [4, D]
        S = r["S"].astype(np.float64)         # [P, NT+4]
        B2 += float((c * V[:3]).sum())
        T2 += float((s_cent * V[3]).sum())
        wsum += S[:, NT:].sum(axis=0)
        # S[p, t] = |f_{row p*NT+t}|^2 for this core's shard; weight on host
        w_pt = wei[i * BC:(i + 1) * BC].astype(np.float64).reshape(P, NT)
        A += float((w_pt * S[:, :NT]).sum())

    own = A - 2.0 * B2 + float((cn * wsum[:3]).sum())
    total = 3.0 * A - 2.0 * T2 + csum * wsum[3]
    distocen = total - own
    out = own * (1.0 + 1.0 / distocen) / 2.0 / bsz
    return np.float32(out)


# revision 17
# speedup vs baseline: 1.2321x; 1.0095x over previous
"""CenterLossB kernel for 8 Trainium2 NeuronCores.

Data-parallel over the batch: each of the 8 cores processes 8192 rows of
feat/label/wei.  The loss

    own      = sum_i w_i * ||f_i - c_{l_i}||^2
    distocen = sum_i w_i * (||f_i - c_{(l_i+1)%3}||^2 + ||f_i - c_{(l_i+2)%3}||^2)
    out      = own * (1 + 1/distocen) / 2 / B

is rewritten algebraically so the device only produces small per-core
partials (centers never need to go to the device):

    A       = sum_i w_i ||f_i||^2                  (ACT Square + row accum)
    V[k,:]  = sum_i w_i 1[l_i=k] f_i   k=0..2      (PE matmul, PSUM fp32 accum)
    V[3,:]  = sum_i w_i f_i
    wsum[k] = sum_i w_i 1[l_i=k],  wsum[3] = sum_i w_i

The device ships per-partition partials (V plus a [128, 68] tile of
per-sub-tile |f|^2 row sums and wk column sums); the host does all
cross-partition reduction and the final combine in float64:
    A     = sum_{p,t} wei[p,t] * rowsq[p,t]
    own   = A - 2*sum_k c_k.V[k] + sum_k ||c_k||^2 wsum[k]
    total = 3A - 2*(sum_k c_k).V[3] + (sum_k ||c_k||^2) * wsum[3]
    distocen = total - own

Engine split while the feat stream DMAs in (4 MiB chunks tapering to
256 KiB so the post-stream compute tail is one sub-tile):
    SP ring   : feat chunks, V out       (the bulk of qSPDynamicHW)
    ACT ring  : wei/label in, S out      (qActDynamicHW)
    PE        : V accumulation (fp32r full-rate matmul)
    DVE + ACT : |f|^2 row sums, alternating sub-tiles (fp32 2-src DVE ops
                are 1x-mode ~626 ns and ACT Square is ~799 ns per sub-tile;
                either alone backlogs behind the ~790 ns/sub-tile stream,
                together they hide completely)
"""

import os
from contextlib import ExitStack

import numpy as np

import concourse.bass as bass
import concourse.bacc as bacc
import concourse.tile as tile
from concourse import mybir
from concourse.bass_utils import run_bass_kernel_spmd

N_CORES = 8
B = 65536
D = 512
BC = B // N_CORES          # 8192 rows per core
P = 128                    # SBUF partitions
NT = BC // P               # 64 sub-tiles of 128 rows

# DMA plan: sub-tiles per dma_start. Large chunks stream at peak HBM rate;
# the tapered tail keeps the last-landing data (and hence the post-stream
# compute tail) small.
SIZES = (16, 16, 8, 8, 8, 4, 2, 1, 1)

# Results of the last device run (for test harnesses to inspect timing).
LAST_RESULTS = None

_COMPILED = {}


def _build(reps=1, sizes=SIZES, bufs=5, stt="mix", dma_only=False,
           io_rings=True, out_ring="io"):
    f32 = mybir.dt.float32
    f32r = mybir.dt.float32r
    i32 = mybir.dt.int32
    X = mybir.AxisListType.X
    op = mybir.AluOpType
    AF = mybir.ActivationFunctionType

    nc = bacc.Bacc("TRN2", target_bir_lowering=False, debug=False,
                   num_devices=N_CORES)

    # feat is declared float32r (same 32-bit layout as fp32; numpy side is
    # np.float32) so the full-rate fp32r matmul path passes BIR verification:
    # the verifier requires producers of fp32r matmul operands to be
    # fp32r-typed.  Non-matmul consumers bitcast back to plain fp32.
    feat_d = nc.dram_tensor("feat", [BC, D], f32r, kind="ExternalInput")
    lab_d = nc.dram_tensor("label", [BC], i32, kind="ExternalInput")
    wei_d = nc.dram_tensor("wei", [BC], f32, kind="ExternalInput")
    # S[p, 0:NT]  = |f_{p,t}|^2 row sums (unweighted; host applies wei)
    # S[p, NT+k]  = sum_t wk[p, k, t]    (host sums over partitions)
    V_d = nc.dram_tensor("V", [4, D], f32, kind="ExternalOutput")
    S_d = nc.dram_tensor("S", [P, NT + 4], f32, kind="ExternalOutput")

    # row layout: global row r = p*NT + t  (p = partition, t = sub-tile idx)
    feat_r = feat_d.ap().rearrange("(p t) d -> p t d", p=P)
    lab_r = lab_d.ap().rearrange("(p t) -> p t", p=P)
    wei_r = wei_d.ap().rearrange("(p t) -> p t", p=P)

    assert sum(sizes) == NT
    starts = [sum(sizes[:i]) for i in range(len(sizes))]
    max_chunk = max(sizes)

    with tile.TileContext(nc) as tc, ExitStack() as ctx:
        singles = ctx.enter_context(tc.tile_pool(name="singles", bufs=1))
        feat_pool = ctx.enter_context(tc.tile_pool(name="feat", bufs=bufs))
        psum = ctx.enter_context(tc.tile_pool(name="psum", bufs=1, space="PSUM"))

        # small I/O rides the second HWDGE ring (qActDynamicHW) so the feat
        # stream on qSPDynamicHW starts without queuing behind it
        io_eng = nc.scalar if io_rings else nc.sync
        w_all = singles.tile([P, NT], f32)
        io_eng.dma_start(w_all[:], wei_r)
        lab_i = singles.tile([P, NT], i32)
        io_eng.dma_start(lab_i[:], lab_r)
        lab_f = singles.tile([P, NT], f32)
        nc.vector.tensor_copy(lab_f[:], lab_i[:])

        # wk[p, k, t] = w * 1[label == k] for k=0..2; wk[p, 3, t] = w
        wk = singles.tile([P, 4, NT], f32)
        for k in range(3):
            nc.vector.scalar_tensor_tensor(
                out=wk[:, k, :], in0=lab_f[:], scalar=float(k), in1=w_all[:],
                op0=op.is_equal, op1=op.mult)
        nc.vector.tensor_copy(wk[:, 3, :], w_all[:])
        # bit-preserving copy into an fp32r-typed tile for the matmul lhsT
        wk_r = singles.tile([P, 4, NT], f32r)
        nc.vector.tensor_copy(wk_r[:], wk[:].bitcast(f32r))
        wk_t = wk_r.rearrange("p k t -> p t k")

        trash = singles.tile([P, D], f32)
        trash2 = singles.tile([P, D], f32)
        # qs[:, 0:NT] = per-sub-tile |f|^2 row sums; qs[:, NT:] = wk sums.
        # The whole [P, NT+4] tile ships to the host in one DMA — no
        # cross-partition reduction on the device's critical path.
        qs = singles.tile([P, NT + 4], f32)
        for k in range(4):
            nc.vector.tensor_reduce(out=qs[:, NT + k:NT + k + 1],
                                    in_=wk[:, k, :], axis=X, op=op.add)

        # which engine squares sub-tile t (DVE takes odd t so the final
        # sub-tile's square runs on the cheaper engine)
        def on_dve(t):
            return stt == "dve" or (stt == "mix" and t % 2 == 1)

        out_eng = nc.sync if out_ring == "sp" else io_eng

        def body():
            psum_V = None
            if not dma_only:
                psum_V = psum.tile([4, D], f32, tag="psum_V")
            for c, (t0, sz) in enumerate(zip(starts, sizes)):
                F = feat_pool.tile([P, max_chunk, D], f32r)
                nc.sync.dma_start(F[:, :sz, :], feat_r[:, t0:t0 + sz, :])
                for a in range(sz if not dma_only else 0):
                    t = t0 + a
                    nc.tensor.matmul(
                        psum_V[:],
                        wk_t[:, t, :],
                        F[:, a, :],
                        start=(t == 0), stop=(t == NT - 1))
                    if on_dve(t):
                        nc.vector.scalar_tensor_tensor(
                            out=trash[:], in0=F[:, a, :].bitcast(f32),
                            scalar=1.0,
                            in1=F[:, a, :].bitcast(f32),
                            op0=op.mult, op1=op.mult,
                            accum_out=qs[:, t:t + 1])
                    else:
                        nc.scalar.activation(
                            out=trash2[:], in_=F[:, a, :].bitcast(f32),
                            func=AF.Square,
                            accum_out=qs[:, t:t + 1])

            if dma_only:
                nc.vector.memset(qs[:, 0:NT], 0.0)
            out_eng.dma_start(S_d.ap(), qs[:])

            v_sb = singles.tile([4, D], f32)
            if dma_only:
                nc.vector.memset(v_sb[:], 0.0)
            elif stt == "dve":
                nc.vector.tensor_copy(v_sb[:], psum_V[:])
            else:
                # DVE squares the final sub-tile; ACT drains V from PSUM
                nc.scalar.activation(out=v_sb[:], in_=psum_V[:], func=AF.Copy)
            # V rides SP (idle once the feat stream ends) while S rides the
            # ACT ring — the two output DMAs land concurrently
            nc.sync.dma_start(V_d.ap(), v_sb[:])

        if reps == 1:
            body()
        else:
            with tc.For_i(0, reps, 1):
                body()

    nc.compile()
    return nc


def _get_compiled(reps=1, **kw):
    key = (reps, tuple(sorted(kw.items())))
    if key not in _COMPILED:
        _COMPILED[key] = _build(reps, **kw)
    return _COMPILED[key]


def kernel(feat, label, wei, centers, batch_size):
    global LAST_RESULTS
    feat = np.ascontiguousarray(np.asarray(feat, dtype=np.float32))
    label = np.ascontiguousarray(np.asarray(label, dtype=np.int32))
    wei = np.ascontiguousarray(np.asarray(wei, dtype=np.float32))
    centers = np.asarray(centers, dtype=np.float32)
    bsz = float(np.asarray(batch_size))

    nc = _get_compiled()

    in_maps = []
    for i in range(N_CORES):
        sl = slice(i * BC, (i + 1) * BC)
        in_maps.append({
            "feat": feat[sl],
            "label": label[sl],
            "wei": wei[sl],
        })

    try:
        res = run_bass_kernel_spmd(nc, in_maps, list(range(N_CORES)))
    except ModuleNotFoundError:
        # BASS_TRACE was requested but this environment lacks the axon NTFF
        # profile hook (antenv.axon_hooks) — rerun without tracing.
        prev = os.environ.get("BASS_NEVER_TRACE")
        os.environ["BASS_NEVER_TRACE"] = "1"
        try:
            res = run_bass_kernel_spmd(nc, in_maps, list(range(N_CORES)))
        finally:
            if prev is None:
                os.environ.pop("BASS_NEVER_TRACE", None)
            else:
                os.environ["BASS_NEVER_TRACE"] = prev
    LAST_RESULTS = res

    # host combine in float64
    c = centers.astype(np.float64)            # [3, D]
    cn = (c * c).sum(axis=1)                  # ||c_k||^2
    csum = cn.sum()
    s_cent = c.sum(axis=0)                    # sum_k c_k
    A = 0.0
    B2 = 0.0
    T2 = 0.0
    wsum = np.zeros(4, dtype=np.float64)
    for i, r in enumerate(res.results):
        V = r["V"].astype(np.float64)         # [4, D]
        S = r["S"].astype(np.float64)         # [P, NT+4]
        B2 += float((c * V[:3]).sum())
        T2 += float((s_cent * V[3]).sum())
        wsum += S[:, NT:].sum(axis=0)
        # S[p, t] = |f_{row p*NT+t}|^2 for this core's shard; weight on host
        w_pt = wei[i * BC:(i + 1) * BC].astype(np.float64).reshape(P, NT)
        A += float((w_pt * S[:, :NT]).sum())

    own = A - 2.0 * B2 + float((cn * wsum[:3]).sum())
    total = 3.0 * A - 2.0 * T2 + csum * wsum[3]
    distocen = total - own
    out = own * (1.0 + 1.0 / distocen) / 2.0 / bsz
    return np.float32(out)
